# revision 35
# baseline (speedup 1.0000x reference)
"""BEiT window attention (B=8, N=1024, C=768, 12 heads) on 8 TRN2 NeuronCores.

Sharding: pure data-parallel over batch - one batch element per core, no
collectives.  v2: single software-pipelined emission.

Per-core dataflow (bf16 matmuls, f32 PSUM accumulation): qT/kT computed
feature-major so attention scores are produced directly as S^T (keys on
partitions) and softmax needs no on-device transposes.  P = exp(S^T) * E^T
with E = exp(rel-pos bias) precomputed host-side.  Softmax denominators come
from an all-ones column appended to each V block (row 64 of the attn@v PSUM
accumulators).

Structure (one software-pipelined emission, no serial phases):
 - PE warmup on junk matmuls while input DMAs stream (the PE clock ramps
   0.65->1.2->2.4 GHz with continuous execution).
 - Only qk chunks 0/6 and V chunks 0-5 run up front; V6/V7 and the ten
   remaining qkv chunks are interleaved into the attention beats as PE
   filler (two 1-bank half-accumulations per chunk) so the tensor engine
   never idles while the Scalar engine runs the 96 exp ACTs (~1.11us each).
 - Per beat: scores -> exp (Scalar) -> *E (DVE) -> attn@v, with attn@v
   skewed SKEW beats behind so its semaphores are satisfied on arrival.
 - Softmax reciprocals ride a spread-DMA/32-lane-reciprocal/DRAM-broadcast
   chain; the normalization multiplies are deferred 5 beats so the DVE
   never blocks on that latency; the last unit's chain uses the idle
   Scalar engine + Sync DMA queue since it gates the projection.
 - Output projection in 3 groups of 2 psum accumulators: pairs 0-4
   accumulate while the last normalization drains, pair-5 closers run the
   moment each op_sb half lands.  bf16 DRAM output halves the final drain.
 - PSUM budget exactly 8 banks: scores 2x2, filler 1x1, o-accum 3x1.
"""

import sys
import types

import numpy as np
import ml_dtypes

BF16NP = ml_dtypes.bfloat16

P = 128        # partitions
NTOK = 1024    # tokens per batch element
C = 768        # embed dim
NH = 12        # heads
HD = 64        # head dim
NPAIR = 6      # head pairs
NQT = 2        # query tiles of 512
QW = 512       # query tile width
KC = 8         # key chunks of 128
NCORES = 8
SKEW = 4       # beats between scores and attn@v consumption
NBEAT = NPAIR * NQT * KC  # 96


def _install_axon_hooks():
    """Register the NTFF profile hook module missing from this image's antenv."""
    if "antenv.axon_hooks" in sys.modules:
        return
    try:
        import antenv  # noqa: F401
        from trn_agent_boot.trn_boot import _ntff_profile_via_ctypes

        mod = types.ModuleType("antenv.axon_hooks")
        mod._hook = _ntff_profile_via_ctypes("/opt/axon/libaxon_pjrt.so")
        mod.get_axon_ntff_profile_hook = lambda: mod._hook
        mod.set_axon_ntff_profile_hook = lambda h: setattr(mod, "_hook", h)
        sys.modules["antenv.axon_hooks"] = mod
    except Exception:
        pass


_BUILD_CACHE = {}


def _build():
    if "nc" in _BUILD_CACHE:
        return _BUILD_CACHE["nc"]

    from contextlib import ExitStack

    import concourse.bass as bass
    import concourse.bacc as bacc
    import concourse.mybir as mybir
    import concourse.tile as tile

    BF = mybir.dt.bfloat16
    F32 = mybir.dt.float32
    AF = mybir.ActivationFunctionType

    nc = bacc.Bacc("TRN2", target_bir_lowering=False, debug=False)

    xT_d = nc.dram_tensor("xT", [C, NTOK], BF, kind="ExternalInput").ap()
    wqkvT_d = nc.dram_tensor("wqkvT", [C, 3 * C], BF, kind="ExternalInput").ap()
    qkb_d = nc.dram_tensor("qkb", [P, 12], F32, kind="ExternalInput").ap()
    vb_d = nc.dram_tensor("vb", [1, C], BF, kind="ExternalInput").ap()
    # (pair, qtile, kchunk, key-part, head-in-pair, q)
    ET_d = nc.dram_tensor(
        "ET", [NPAIR, NQT, KC, P, 2, QW], BF, kind="ExternalInput"
    ).ap()
    pwT_d = nc.dram_tensor("pwT", [C, C], BF, kind="ExternalInput").ap()
    pbT_d = nc.dram_tensor("pbT", [P, 6], F32, kind="ExternalInput").ap()
    out_d = nc.dram_tensor("out", [C, NTOK], BF, kind="ExternalOutput").ap()

    units = [(p, qt) for p in range(NPAIR) for qt in range(NQT)]

    with ExitStack() as ctx:
        tc = ctx.enter_context(tile.TileContext(nc))
        const = ctx.enter_context(tc.tile_pool(name="const", bufs=1))
        spool = ctx.enter_context(tc.tile_pool(name="spool", bufs=2, space="PSUM"))
        qkpool = ctx.enter_context(tc.tile_pool(name="qkps", bufs=1, space="PSUM"))
        opool = ctx.enter_context(tc.tile_pool(name="opool", bufs=3, space="PSUM"))
        epool = ctx.enter_context(tc.tile_pool(name="epool", bufs=5))
        prawp = ctx.enter_context(tc.tile_pool(name="praw", bufs=6))
        pfinp = ctx.enter_context(tc.tile_pool(name="pfin", bufs=6))
        ocpp = ctx.enter_context(tc.tile_pool(name="ocp", bufs=2))
        smallp = ctx.enter_context(tc.tile_pool(name="small", bufs=4))
        sinvp = ctx.enter_context(tc.tile_pool(name="sinv", bufs=2))
        stgp = ctx.enter_context(tc.tile_pool(name="stg", bufs=2))
        dramp = ctx.enter_context(tc.tile_pool(name="dram", bufs=4, space="DRAM"))
        youtp = ctx.enter_context(tc.tile_pool(name="yout", bufs=2))

        # ---- persistent SBUF tensors ----
        xT_sb = const.tile([P, 6, NTOK], BF)          # x^T, feature-major
        w_sb = const.tile([P, 6, 3 * C], BF)          # qkv_w^T (q cols pre-scaled)
        qk_sb = const.tile([P, 12, NTOK], BF)         # q^T (chunks 0-5), k^T (6-11)
        v_sb = const.tile([P, KC, NH * (HD + 1)], BF)  # 12x[v_h|1] blocks per kchunk
        op_sb = const.tile([P, NPAIR, NTOK], BF)      # normalized O^T, pair-stacked
        pw_sb = const.tile([P, 6, C], BF)             # proj_w^T
        qkb_sb = const.tile([P, 12], F32)
        pb_sb = const.tile([P, 6], F32)
        vb_sb = const.tile([P, C], BF)

        # ---- input DMAs, priority order (single in-order SP queue) ----
        # half-granularity so the first qk matmuls can start before the full
        # weight set lands
        x_r = xT_d.rearrange("(c p) n -> p c n", p=P)
        w_r = wqkvT_d.rearrange("(c p) m -> p c m", p=P)
        for c in range(6):
            nc.sync.dma_start(out=w_sb[:, c, 0:C], in_=w_r[:, c, 0:C])
            nc.sync.dma_start(out=xT_sb[:, c, :], in_=x_r[:, c, :])
        nc.sync.dma_start(out=qkb_sb[:], in_=qkb_d[:])
        nc.sync.dma_start(out=w_sb[:, 0:3, C:2 * C], in_=w_r[:, 0:3, C:2 * C])
        nc.sync.dma_start(out=w_sb[:, 3:6, C:2 * C], in_=w_r[:, 3:6, C:2 * C])
        nc.sync.dma_start(out=vb_sb[:], in_=vb_d.broadcast_to((P, C)))
        nc.sync.dma_start(out=w_sb[:, 0:3, 2 * C:3 * C], in_=w_r[:, 0:3, 2 * C:3 * C])
        nc.sync.dma_start(out=w_sb[:, 3:6, 2 * C:3 * C], in_=w_r[:, 3:6, 2 * C:3 * C])

        # ones columns of the V blocks (softmax denominator trick)
        nc.gpsimd.memset(
            v_sb[:].rearrange("p k (h w) -> p k h w", w=HD + 1)[:, :, :, HD:HD + 1],
            1.0,
        )

        # PE warmup: the tensor engine clock ramps with continuous execution
        # (0.65 -> 1.2 -> 2.4 GHz after ~3us).  Chew on junk matmuls while the
        # input DMAs stream so the real qkv work starts at full clock.
        junk = const.tile([P, QW], BF)
        nc.gpsimd.memset(junk[:], 0.0)
        for _ in range(20):
            wps = spool.tile([P, NTOK], F32, tag="spool", name="warmps")
            nc.tensor.matmul(
                wps[:, 0:QW], lhsT=junk[:, 0:P], rhs=junk[:],
                start=True, stop=True,
            )

        # ---- building blocks ----
        def qk_matmul_ops(j, pool, cell, tag):
            ops = []

            def get_tile():
                if "ps" not in cell:
                    cell["ps"] = pool.tile(
                        [P, NTOK], F32, tag=tag, name=tag
                    )
                return cell["ps"]

            for c in range(6):
                for half in range(2):
                    def op(c=c, half=half):
                        ps = get_tile()
                        sl = slice(half * QW, (half + 1) * QW)
                        nc.tensor.matmul(
                            ps[:, sl],
                            lhsT=w_sb[:, c, j * P:(j + 1) * P],
                            rhs=xT_sb[:, c, sl],
                            start=(c == 0),
                            stop=(c == 5),
                        )
                    ops.append(op)
            return ops

        def qk_copy_scalar(j, cell):
            # upfront only: Scalar is idle before the exp chain starts
            nc.scalar.activation(
                out=qk_sb[:, j, :], in_=cell["ps"][:], func=AF.Identity,
                bias=qkb_sb[:, j:j + 1],
            )

        def qk_copy_vector(j, cell):
            nc.vector.tensor_scalar_add(
                qk_sb[:, j, :], cell["ps"][:], qkb_sb[:, j:j + 1]
            )

        def v_matmul_ops(t, pool, cell, tag):
            ops = []

            def get_tile():
                if "ps" not in cell:
                    cell["ps"] = pool.tile(
                        [P, NTOK], F32, tag=tag, name=tag
                    )
                return cell["ps"]

            for c in range(6):
                for off, width in ((0, 512), (512, 256)):
                    def op(c=c, off=off, width=width):
                        ps = get_tile()
                        nc.tensor.matmul(
                            ps[:, off:off + width],
                            lhsT=xT_sb[:, c, t * P:(t + 1) * P],
                            rhs=w_sb[:, c, 2 * C + off:2 * C + off + width],
                            start=(c == 0),
                            stop=(c == 5),
                        )
                    ops.append(op)
            return ops

        def v_add(t, cell):
            nc.vector.tensor_add(
                v_sb[:, t, :].rearrange("p (h w) -> p h w", w=HD + 1)[:, :, 0:HD],
                cell["ps"][:, 0:C].rearrange("p (h w) -> p h w", w=HD),
                vb_sb[:].rearrange("p (h w) -> p h w", w=HD),
            )

        # ---- flat attention loop: 96 beats + SKEW trailing ----
        e_tiles = {}
        ptiles = {}
        unit_state = {}
        deferred = {}  # global beat -> [callables]

        def issue_e(g):
            p, qt = units[g // 8]
            kc = g % 8
            e_sb = epool.tile([P, NTOK], BF, tag="e")
            nc.sync.dma_start(
                out=e_sb[:],
                in_=ET_d[p:p + 1, qt:qt + 1, kc:kc + 1, :, :, :].rearrange(
                    "a b c p h q -> (a p) (b c h q)"
                ),
            )
            e_tiles[g] = e_sb

        def score_beat(g):
            p, qt = units[g // 8]
            kc = g % 8
            # scores: S^T tile [128 keys, 2 heads x 512 queries]
            s_ps = spool.tile([P, NTOK], F32, tag="spool", name="s_ps")
            for h in range(2):
                hh = HD * h
                nc.tensor.matmul(
                    s_ps[:, h * QW:(h + 1) * QW],
                    lhsT=qk_sb[hh:hh + HD, 6 + p, kc * P:(kc + 1) * P],
                    rhs=qk_sb[hh:hh + HD, p, qt * QW:(qt + 1) * QW],
                    start=True,
                    stop=True,
                )
            praw = prawp.tile([P, NTOK], BF, tag="praw", name="praw")
            nc.scalar.activation(out=praw[:], in_=s_ps[:], func=AF.Exp)
            ptile = pfinp.tile([P, NTOK], BF, tag="pfin", name="ptile")
            nc.vector.tensor_mul(ptile[:], praw[:], e_tiles.pop(g)[:])
            ptiles[g] = ptile


        # ---- upfront: qk(0), qk(6), V(0..5) ----
        PRE = 0
        for j in (0, 6):
            cell = {}
            for op in qk_matmul_ops(j, spool, cell, "spool"):
                op()
            qk_copy_scalar(j, cell)
        for t in range(6):
            cell = {}
            for op in v_matmul_ops(t, spool, cell, "spool"):
                op()
            v_add(t, cell)
        issue_e(0)
        issue_e(1)

        # ---- fill plan: PE work interleaved into the attention beats ----
        # each chunk is two half-accumulations through a single 1-bank psum
        # tile, so the filler pool costs 1 bank and the freed bank deepens
        # the o-accumulator pool
        def fill_tile(cell):
            if "ps" not in cell:
                cell["ps"] = qkpool.tile(
                    [P, QW], F32, tag="fillps", name="fillps"
                )
            return cell["ps"]

        def make_qk_fill(j):
            ops = []
            for half in range(2):
                cell = {}
                for c in range(6):
                    def op(c=c, half=half, cell=cell):
                        ps = fill_tile(cell)
                        nc.tensor.matmul(
                            ps[:],
                            lhsT=w_sb[:, c, j * P:(j + 1) * P],
                            rhs=xT_sb[:, c, half * QW:(half + 1) * QW],
                            start=(c == 0),
                            stop=(c == 5),
                        )
                    ops.append(op)

                def copy_op(half=half, cell=cell):
                    nc.vector.tensor_scalar_add(
                        qk_sb[:, j, half * QW:(half + 1) * QW],
                        cell["ps"][:],
                        qkb_sb[:, j:j + 1],
                    )
                ops.append(copy_op)
            return ops

        def make_v_fill(t):
            ops = []
            for half in range(2):
                cell = {}
                width = 512 if half == 0 else 256
                for c in range(6):
                    def op(c=c, half=half, width=width, cell=cell):
                        ps = fill_tile(cell)
                        off = 2 * C + half * QW
                        nc.tensor.matmul(
                            ps[:, 0:width],
                            lhsT=xT_sb[:, c, t * P:(t + 1) * P],
                            rhs=w_sb[:, c, off:off + width],
                            start=(c == 0),
                            stop=(c == 5),
                        )
                    ops.append(op)

                def add_op(half=half, width=width, cell=cell):
                    nh0 = half * 8
                    nh = width // HD
                    nc.vector.tensor_add(
                        v_sb[:, t, :].rearrange(
                            "p (h w) -> p h w", w=HD + 1
                        )[:, nh0:nh0 + nh, 0:HD],
                        cell["ps"][:, 0:width].rearrange(
                            "p (h w) -> p h w", w=HD
                        ),
                        vb_sb[:, half * QW:half * QW + width].rearrange(
                            "p (h w) -> p h w", w=HD
                        ),
                    )
                ops.append(add_op)
            return ops

        fill_plan = {
            0: [("v", 6), ("v", 7)],
            1: [("qk", 7), ("qk", 1)],
            2: [("qk", 8)], 3: [("qk", 2)],
            4: [("qk", 9)], 5: [("qk", 3)],
            6: [("qk", 10)], 7: [("qk", 4)],
            8: [("qk", 11)], 9: [("qk", 5)],
            10: [], 11: [],
        }
        fills_by_beat = {}  # global beat -> [callables]
        for u, chunks in fill_plan.items():
            ops = []
            for kind, idx in chunks:
                ops.extend(make_v_fill(idx) if kind == "v" else make_qk_fill(idx))
            if not ops:
                continue
            per_beat = -(-len(ops) // 8)
            for i, op in enumerate(ops):
                g = u * 8 + min(i // per_beat, 7)
                fills_by_beat.setdefault(g, []).append(op)

        def norm_a(u):
            """Evacuate o accumulators + start the reciprocal/broadcast chain."""
            st = unit_state[u]
            ocp = ocpp.tile([P, NTOK], F32, tag="ocp")
            if u == len(units) - 1:
                # the exp chain is over: the idle Scalar engine beats the
                # backed-up DVE queue to free the accumulators that gate the
                # output projection's pair-5 step
                nc.scalar.copy(ocp[0:HD + 1, 0:QW], st["oA"][0:HD + 1, :])
                nc.scalar.copy(ocp[0:HD + 1, QW:NTOK], st["oB"][0:HD + 1, :])
            else:
                nc.vector.tensor_copy(ocp[0:HD + 1, 0:QW], st["oA"][0:HD + 1, :])
                nc.vector.tensor_copy(ocp[0:HD + 1, QW:NTOK], st["oB"][0:HD + 1, :])
            # spread the 1024 sums over 8 partitions so the iterative divide
            # runs 8 lanes wide
            dma = nc.sync if u == len(units) - 1 else nc.gpsimd
            rsp = smallp.tile([32, 32], F32, tag="rsp")
            dma.dma_start(out=rsp[:], in_=ocp[HD:HD + 1, :])
            rinv = smallp.tile([32, 32], F32, tag="rinv")
            nc.vector.reciprocal(rinv[:], rsp[:])
            dscr = dramp.tile([1, NTOK], F32, tag="ds")
            dma.dma_start(out=dscr[:], in_=rinv[:])
            sinv = sinvp.tile([HD, NTOK], F32, tag="sinv")
            dma.dma_start(
                out=sinv[:], in_=dscr[:].broadcast_to((HD, NTOK))
            )
            st["ocp"] = ocp
            st["sinv"] = sinv

        def norm_b(u):
            """Normalize into op_sb (deferred so the DVE never blocks on the
            reciprocal/broadcast latency chain)."""
            p, qt = units[u]
            st = unit_state[u]
            ocp = st["ocp"]
            # the broadcast and the h1 partition-shift ride the idle Sync
            # queue for the last two units so the projection's pair-5 gate
            # isn't stuck behind the congested gpsimd queue
            sinv = st["sinv"]
            dma = nc.sync if u == len(units) - 1 else nc.gpsimd
            nc.vector.tensor_mul(
                op_sb[0:HD, p, qt * QW:(qt + 1) * QW],
                ocp[0:HD, 0:QW],
                sinv[:, 0:QW],
            )
            stage = stgp.tile([HD, QW], BF, tag="stage")
            nc.vector.tensor_mul(stage[:], ocp[0:HD, QW:NTOK], sinv[:, QW:NTOK])
            dma.dma_start(
                out=op_sb[HD:P, p, qt * QW:(qt + 1) * QW], in_=stage[:]
            )

        def av_beat(b):
            u = b // 8
            kcb = b % 8
            if kcb == 0:
                oA = opool.tile([P, QW], F32, tag="o", name="oA")
                oB = opool.tile([P, QW], F32, tag="o", name="oB")
                unit_state[u] = {"oA": oA, "oB": oB}
            st = unit_state[u]
            pt = ptiles.pop(b)
            pu, _ = units[u]
            for h in range(2):
                head = 2 * pu + h
                o_ps = st["oA"] if h == 0 else st["oB"]
                nc.tensor.matmul(
                    o_ps[0:HD + 1, :],
                    lhsT=v_sb[:, kcb, (HD + 1) * head:(HD + 1) * (head + 1)],
                    rhs=pt[:, h * QW:(h + 1) * QW],
                    start=(kcb == 0),
                    stop=(kcb == KC - 1),
                )
            return u if kcb == KC - 1 else None

        for g in range(NBEAT + SKEW):
            for op in deferred.pop(g, ()):
                op()
            if g < NBEAT:
                if g + 2 < NBEAT:
                    issue_e(g + 2)
                score_beat(g)
                for op in fills_by_beat.pop(g, ()):
                    op()
            b = g - SKEW
            if b >= 0:
                udone = av_beat(b)
                if udone is not None:
                    norm_a(udone)
                    deferred.setdefault(g + 5, []).append(
                        lambda u=udone: norm_b(u)
                    )
            if g == 72:
                # proj weights, needed from ~t=150us; issue mid-stream
                nc.sync.dma_start(
                    out=pw_sb[:], in_=pwT_d.rearrange("(c p) m -> p c m", p=P)
                )
                nc.sync.dma_start(out=pb_sb[:], in_=pbT_d[:])
        for ops in deferred.values():
            for op in ops:
                op()

        # ---- output projection ----
        # two groups of 3 psum accumulators; pairs 0-4 accumulate while the
        # last unit's normalization chain drains, pair 5 appended once its
        # op_sb lands, so the PE never idles behind the norm latency
        def proj_alloc_spool():
            ta = spool.tile([P, NTOK], F32, tag="spool", name="projps")
            tb = spool.tile([P, NTOK], F32, tag="spool", name="projps")
            return [
                (ta[:, 0:QW], ta[:, QW:NTOK]),
                (tb[:, 0:QW], tb[:, QW:NTOK]),
            ]

        def proj_partial(ec, halves):
            for p in range(5):
                for nt in range(2):
                    nc.tensor.matmul(
                        halves[nt],
                        lhsT=pw_sb[:, p, ec * P:(ec + 1) * P],
                        rhs=op_sb[:, p, nt * QW:(nt + 1) * QW],
                        start=(p == 0),
                        stop=False,
                    )

        def proj_close(ec, halves, nt):
            nc.tensor.matmul(
                halves[nt],
                lhsT=pw_sb[:, 5, ec * P:(ec + 1) * P],
                rhs=op_sb[:, 5, nt * QW:(nt + 1) * QW],
                start=False,
                stop=True,
            )

        def proj_evac(ec, halves):
            # Scalar and DVE each take one half: both are idle at the tail,
            # halving the bias-add chain; output DMAs alternate queues
            y_sb = youtp.tile([P, NTOK], BF, tag="y", name="y_sb")
            nc.scalar.activation(
                out=y_sb[:, 0:QW], in_=halves[0], func=AF.Identity,
                bias=pb_sb[:, ec:ec + 1],
            )
            nc.vector.tensor_scalar_add(
                y_sb[:, QW:NTOK], halves[1], pb_sb[:, ec:ec + 1]
            )
            dma = nc.sync if ec % 2 == 0 else nc.gpsimd
            dma.dma_start(out=out_d[ec * P:(ec + 1) * P, :], in_=y_sb[:])

        # groups A (spool) and B (the four 1-bank tiles) accumulate pairs 0-4
        # while the last normalization chain drains; every pair-5 closer runs
        # as soon as its op_sb half lands; group C reuses spool afterwards
        ga = proj_alloc_spool()
        for i, ec in enumerate((0, 1)):
            proj_partial(ec, ga[i])
        th0 = qkpool.tile([P, QW], F32, tag="fillps", name="projh0")
        to = [opool.tile([P, QW], F32, tag="o", name="projh") for _ in range(3)]
        gb = [(th0[:], to[0][:]), (to[1][:], to[2][:])]
        for i, ec in enumerate((2, 3)):
            proj_partial(ec, gb[i])
        for nt in range(2):
            for i in range(2):
                proj_close(i, ga[i], nt)
                proj_close(2 + i, gb[i], nt)
        for i in range(2):
            proj_evac(i, ga[i])
            proj_evac(2 + i, gb[i])
        gc = proj_alloc_spool()
        for i, ec in enumerate((4, 5)):
            proj_partial(ec, gc[i])
            for nt in range(2):
                proj_close(ec, gc[i], nt)
            proj_evac(ec, gc[i])


    nc.compile()
    _BUILD_CACHE["nc"] = nc
    return nc


def _prep_inputs(x, qkv_w, q_bias, v_bias, rel_bias_table, proj_w, proj_b,
                 rel_pos_idx):
    x = np.asarray(x, np.float32)
    qkv_w = np.asarray(qkv_w, np.float32)
    q_bias = np.asarray(q_bias, np.float32)
    v_bias = np.asarray(v_bias, np.float32)
    rel_bias_table = np.asarray(rel_bias_table, np.float32)
    proj_w = np.asarray(proj_w, np.float32)
    proj_b = np.asarray(proj_b, np.float32)
    rel_pos_idx = np.asarray(rel_pos_idx, np.int64)

    scale = HD ** -0.5
    wq = qkv_w[:C] * scale
    wqkvT = np.ascontiguousarray(
        np.concatenate([wq, qkv_w[C:]], axis=0).T
    ).astype(BF16NP)

    qk_bias = np.concatenate([q_bias * scale, np.zeros(C, np.float32)])
    qkb = np.ascontiguousarray(qk_bias.reshape(12, P).T)

    vb = v_bias.astype(BF16NP).reshape(1, C)

    # E^T[h, m, n] = exp(bias[h, n, m]); bias[h, n, m] = table[idx[n, m], h]
    A = np.exp(rel_bias_table)[rel_pos_idx]            # (n, m, h)
    ETpre = A.transpose(2, 1, 0)                       # (h, m, n)
    ET = np.ascontiguousarray(
        ETpre.reshape(NPAIR, 2, KC, P, NQT, QW).transpose(0, 4, 2, 3, 1, 5)
    ).astype(BF16NP)

    pwT = np.ascontiguousarray(proj_w.T).astype(BF16NP)
    pbT = np.ascontiguousarray(proj_b.reshape(6, P).T)

    shared = {
        "wqkvT": wqkvT, "qkb": qkb, "vb": vb, "ET": ET,
        "pwT": pwT, "pbT": pbT,
    }
    in_maps = []
    xb16 = x.reshape(NCORES, NTOK, C).astype(BF16NP)
    for b in range(NCORES):
        m = dict(shared)
        m["xT"] = np.ascontiguousarray(xb16[b].T)
        in_maps.append(m)
    return in_maps


def _run(inputs, trace=False):
    import time as _time

    _install_axon_hooks()
    from concourse.bass_utils import run_bass_kernel_spmd

    t0 = _time.time()
    nc = _build()
    print(f"[kernel] build+compile: {_time.time() - t0:.1f}s", flush=True)
    t0 = _time.time()
    in_maps = _prep_inputs(**inputs)
    print(f"[kernel] host prep: {_time.time() - t0:.1f}s", flush=True)
    t0 = _time.time()
    res = run_bass_kernel_spmd(
        nc, in_maps, core_ids=list(range(NCORES)), trace=trace
    )
    print(f"[kernel] hw run: {_time.time() - t0:.1f}s", flush=True)
    outs = [np.asarray(res.results[b]["out"]) for b in range(NCORES)]
    y = np.stack([o.T.reshape(32, 32, C) for o in outs]).astype(np.float32)
    return y, res


def kernel(**inputs) -> np.ndarray:
    y, _ = _run(inputs, trace=False)
    return y


# revision 36
# speedup vs baseline: 1.0928x; 1.0928x over previous
"""BEiT window attention (B=8, N=1024, C=768, 12 heads) on 8 TRN2 NeuronCores.

Sharding: pure data-parallel over batch - one batch element per core, no
collectives.  v2: single software-pipelined emission.

Per-core dataflow (bf16 matmuls, f32 PSUM accumulation): qT/kT computed
feature-major so attention scores are produced directly as S^T (keys on
partitions) and softmax needs no on-device transposes.  P = exp(S^T) * E^T
with E = exp(rel-pos bias) precomputed host-side.  Softmax denominators come
from an all-ones column appended to each V block (row 64 of the attn@v PSUM
accumulators).

Structure (one software-pipelined emission, no serial phases):
 - PE warmup on junk matmuls while input DMAs stream (the PE clock ramps
   0.65->1.2->2.4 GHz with continuous execution).
 - Only qk chunks 0/6 and V chunks 0-5 run up front; V6/V7 and the ten
   remaining qkv chunks are interleaved into the attention beats as PE
   filler (two 1-bank half-accumulations per chunk) so the tensor engine
   never idles while the Scalar engine runs the 96 exp ACTs (~1.11us each).
 - Per beat: scores -> exp (Scalar) -> *E (DVE) -> attn@v, with attn@v
   skewed SKEW beats behind so its semaphores are satisfied on arrival.
 - Softmax reciprocals ride a spread-DMA/32-lane-reciprocal/DRAM-broadcast
   chain; the normalization multiplies are deferred 5 beats so the DVE
   never blocks on that latency; the last unit's chain uses the idle
   Scalar engine + Sync DMA queue since it gates the projection.
 - Output projection in 3 groups of 2 psum accumulators: pairs 0-4
   accumulate while the last normalization drains, pair-5 closers run the
   moment each op_sb half lands.  bf16 DRAM output halves the final drain.
 - PSUM budget exactly 8 banks: scores 2x2, filler 1x1, o-accum 3x1.
"""

import sys
import types

import numpy as np
import ml_dtypes

BF16NP = ml_dtypes.bfloat16

P = 128        # partitions
NTOK = 1024    # tokens per batch element
C = 768        # embed dim
NH = 12        # heads
HD = 64        # head dim
NPAIR = 6      # head pairs
NQT = 2        # query tiles of 512
QW = 512       # query tile width
KC = 8         # key chunks of 128
NCORES = 8
SKEW = 4       # beats between scores and attn@v consumption
NBEAT = NPAIR * NQT * KC  # 96


def _install_axon_hooks():
    """Register the NTFF profile hook module missing from this image's antenv."""
    if "antenv.axon_hooks" in sys.modules:
        return
    try:
        import antenv  # noqa: F401
        from trn_agent_boot.trn_boot import _ntff_profile_via_ctypes

        mod = types.ModuleType("antenv.axon_hooks")
        mod._hook = _ntff_profile_via_ctypes("/opt/axon/libaxon_pjrt.so")
        mod.get_axon_ntff_profile_hook = lambda: mod._hook
        mod.set_axon_ntff_profile_hook = lambda h: setattr(mod, "_hook", h)
        sys.modules["antenv.axon_hooks"] = mod
    except Exception:
        pass


_BUILD_CACHE = {}


def _build():
    if "nc" in _BUILD_CACHE:
        return _BUILD_CACHE["nc"]

    from contextlib import ExitStack

    import concourse.bass as bass
    import concourse.bacc as bacc
    import concourse.mybir as mybir
    import concourse.tile as tile

    BF = mybir.dt.bfloat16
    F32 = mybir.dt.float32
    AF = mybir.ActivationFunctionType

    nc = bacc.Bacc("TRN2", target_bir_lowering=False, debug=False)

    xT_d = nc.dram_tensor("xT", [C, NTOK], BF, kind="ExternalInput").ap()
    wqkvT_d = nc.dram_tensor("wqkvT", [C, 3 * C], BF, kind="ExternalInput").ap()
    qkb_d = nc.dram_tensor("qkb", [P, 12], F32, kind="ExternalInput").ap()
    vb_d = nc.dram_tensor("vb", [1, C], BF, kind="ExternalInput").ap()
    # (pair, qtile, kchunk, key-part, head-in-pair, q)
    ET_d = nc.dram_tensor(
        "ET", [NPAIR, NQT, KC, P, 2, QW], BF, kind="ExternalInput"
    ).ap()
    pwT_d = nc.dram_tensor("pwT", [C, C], BF, kind="ExternalInput").ap()
    pbT_d = nc.dram_tensor("pbT", [P, 6], F32, kind="ExternalInput").ap()
    out_d = nc.dram_tensor("out", [C, NTOK], BF, kind="ExternalOutput").ap()

    units = [(p, qt) for p in range(NPAIR) for qt in range(NQT)]

    with ExitStack() as ctx:
        tc = ctx.enter_context(tile.TileContext(nc))
        const = ctx.enter_context(tc.tile_pool(name="const", bufs=1))
        spool = ctx.enter_context(tc.tile_pool(name="spool", bufs=2, space="PSUM"))
        qkpool = ctx.enter_context(tc.tile_pool(name="qkps", bufs=1, space="PSUM"))
        opool = ctx.enter_context(tc.tile_pool(name="opool", bufs=3, space="PSUM"))
        epool = ctx.enter_context(tc.tile_pool(name="epool", bufs=5))
        prawp = ctx.enter_context(tc.tile_pool(name="praw", bufs=5))
        pfinp = ctx.enter_context(tc.tile_pool(name="pfin", bufs=5))
        ocpp = ctx.enter_context(tc.tile_pool(name="ocp", bufs=2))
        smallp = ctx.enter_context(tc.tile_pool(name="small", bufs=4))
        sinvp = ctx.enter_context(tc.tile_pool(name="sinv", bufs=2))
        stgp = ctx.enter_context(tc.tile_pool(name="stg", bufs=2))
        dramp = ctx.enter_context(tc.tile_pool(name="dram", bufs=4, space="DRAM"))
        youtp = ctx.enter_context(tc.tile_pool(name="yout", bufs=2))

        # ---- persistent SBUF tensors ----
        xT_sb = const.tile([P, 6, NTOK], BF)          # x^T, feature-major
        w_sb = const.tile([P, 6, 3 * C], BF)          # qkv_w^T (q cols pre-scaled)
        qk_sb = const.tile([P, 12, NTOK], BF)         # q^T (chunks 0-5), k^T (6-11)
        v_sb = const.tile([P, KC, NH * (HD + 1)], BF)  # 12x[v_h|1] blocks per kchunk
        op_sb = const.tile([P, NPAIR, NTOK], BF)      # normalized O^T, pair-stacked
        pw_sb = const.tile([P, 6, C], BF)             # proj_w^T
        qkb_sb = const.tile([P, 12], F32)
        pb_sb = const.tile([P, 6], F32)
        vb_sb = const.tile([P, C], BF)

        # ---- input DMAs, priority order (single in-order SP queue) ----
        # half-granularity so the first qk matmuls can start before the full
        # weight set lands
        x_r = xT_d.rearrange("(c p) n -> p c n", p=P)
        w_r = wqkvT_d.rearrange("(c p) m -> p c m", p=P)
        for c in range(6):
            nc.sync.dma_start(out=w_sb[:, c, 0:C], in_=w_r[:, c, 0:C])
            nc.sync.dma_start(out=xT_sb[:, c, :], in_=x_r[:, c, :])
        nc.sync.dma_start(out=qkb_sb[:], in_=qkb_d[:])
        nc.sync.dma_start(out=w_sb[:, 0:3, C:2 * C], in_=w_r[:, 0:3, C:2 * C])
        nc.sync.dma_start(out=w_sb[:, 3:6, C:2 * C], in_=w_r[:, 3:6, C:2 * C])
        nc.sync.dma_start(out=vb_sb[:], in_=vb_d.broadcast_to((P, C)))
        nc.sync.dma_start(out=w_sb[:, 0:3, 2 * C:3 * C], in_=w_r[:, 0:3, 2 * C:3 * C])
        nc.sync.dma_start(out=w_sb[:, 3:6, 2 * C:3 * C], in_=w_r[:, 3:6, 2 * C:3 * C])

        # ones columns of the V blocks (softmax denominator trick)
        nc.gpsimd.memset(
            v_sb[:].rearrange("p k (h w) -> p k h w", w=HD + 1)[:, :, :, HD:HD + 1],
            1.0,
        )

        # PE warmup: the tensor engine clock ramps with continuous execution
        # (0.65 -> 1.2 -> 2.4 GHz after ~3us).  Chew on junk matmuls while the
        # input DMAs stream so the real qkv work starts at full clock.
        junk = const.tile([P, QW], BF)
        nc.gpsimd.memset(junk[:], 0.0)
        for _ in range(20):
            wps = spool.tile([P, NTOK], F32, tag="spool", name="warmps")
            nc.tensor.matmul(
                wps[:, 0:QW], lhsT=junk[:, 0:P], rhs=junk[:],
                start=True, stop=True,
            )

        # ---- building blocks ----
        def qk_matmul_ops(j, pool, cell, tag):
            ops = []

            def get_tile():
                if "ps" not in cell:
                    cell["ps"] = pool.tile(
                        [P, NTOK], F32, tag=tag, name=tag
                    )
                return cell["ps"]

            for c in range(6):
                for half in range(2):
                    def op(c=c, half=half):
                        ps = get_tile()
                        sl = slice(half * QW, (half + 1) * QW)
                        nc.tensor.matmul(
                            ps[:, sl],
                            lhsT=w_sb[:, c, j * P:(j + 1) * P],
                            rhs=xT_sb[:, c, sl],
                            start=(c == 0),
                            stop=(c == 5),
                        )
                    ops.append(op)
            return ops

        def qk_copy_scalar(j, cell):
            # upfront only: Scalar is idle before the exp chain starts
            nc.scalar.activation(
                out=qk_sb[:, j, :], in_=cell["ps"][:], func=AF.Identity,
                bias=qkb_sb[:, j:j + 1],
            )

        def qk_copy_vector(j, cell):
            nc.vector.tensor_scalar_add(
                qk_sb[:, j, :], cell["ps"][:], qkb_sb[:, j:j + 1]
            )

        def v_matmul_ops(t, pool, cell, tag):
            ops = []

            def get_tile():
                if "ps" not in cell:
                    cell["ps"] = pool.tile(
                        [P, NTOK], F32, tag=tag, name=tag
                    )
                return cell["ps"]

            for c in range(6):
                for off, width in ((0, 512), (512, 256)):
                    def op(c=c, off=off, width=width):
                        ps = get_tile()
                        nc.tensor.matmul(
                            ps[:, off:off + width],
                            lhsT=xT_sb[:, c, t * P:(t + 1) * P],
                            rhs=w_sb[:, c, 2 * C + off:2 * C + off + width],
                            start=(c == 0),
                            stop=(c == 5),
                        )
                    ops.append(op)
            return ops

        def v_add(t, cell):
            nc.vector.tensor_add(
                v_sb[:, t, :].rearrange("p (h w) -> p h w", w=HD + 1)[:, :, 0:HD],
                cell["ps"][:, 0:C].rearrange("p (h w) -> p h w", w=HD),
                vb_sb[:].rearrange("p (h w) -> p h w", w=HD),
            )

        # ---- flat attention loop: 96 beats + SKEW trailing ----
        e_tiles = {}
        ptiles = {}
        unit_state = {}
        deferred = {}  # global beat -> [callables]

        def issue_e(g):
            p, qt = units[g // 8]
            kc = g % 8
            e_sb = epool.tile([P, NTOK], BF, tag="e")
            nc.sync.dma_start(
                out=e_sb[:],
                in_=ET_d[p:p + 1, qt:qt + 1, kc:kc + 1, :, :, :].rearrange(
                    "a b c p h q -> (a p) (b c h q)"
                ),
            )
            e_tiles[g] = e_sb

        def score_beat(g):
            p, qt = units[g // 8]
            kc = g % 8
            # scores: S^T tile [128 keys, 2 heads x 512 queries]
            s_ps = spool.tile([P, NTOK], F32, tag="spool", name="s_ps")
            for h in range(2):
                hh = HD * h
                nc.tensor.matmul(
                    s_ps[:, h * QW:(h + 1) * QW],
                    lhsT=qk_sb[hh:hh + HD, 6 + p, kc * P:(kc + 1) * P],
                    rhs=qk_sb[hh:hh + HD, p, qt * QW:(qt + 1) * QW],
                    start=True,
                    stop=True,
                )
            praw = prawp.tile([P, NTOK], BF, tag="praw", name="praw")
            nc.scalar.activation(out=praw[:], in_=s_ps[:], func=AF.Exp)
            ptile = pfinp.tile([P, NTOK], BF, tag="pfin", name="ptile")
            nc.vector.tensor_mul(ptile[:], praw[:], e_tiles.pop(g)[:])
            ptiles[g] = ptile


        # ---- upfront: qk(0), qk(6), V(0..5) ----
        PRE = 0
        for j in (0, 6):
            cell = {}
            for op in qk_matmul_ops(j, spool, cell, "spool"):
                op()
            qk_copy_scalar(j, cell)
        for t in range(6):
            cell = {}
            for op in v_matmul_ops(t, spool, cell, "spool"):
                op()
            v_add(t, cell)
        issue_e(0)
        issue_e(1)

        # ---- fill plan: PE work interleaved into the attention beats ----
        # each chunk is two half-accumulations through a single 1-bank psum
        # tile, so the filler pool costs 1 bank and the freed bank deepens
        # the o-accumulator pool
        def fill_tile(cell):
            if "ps" not in cell:
                cell["ps"] = qkpool.tile(
                    [P, QW], F32, tag="fillps", name="fillps"
                )
            return cell["ps"]

        def make_qk_fill(j):
            ops = []
            for half in range(2):
                cell = {}
                for c in range(6):
                    def op(c=c, half=half, cell=cell):
                        ps = fill_tile(cell)
                        nc.tensor.matmul(
                            ps[:],
                            lhsT=w_sb[:, c, j * P:(j + 1) * P],
                            rhs=xT_sb[:, c, half * QW:(half + 1) * QW],
                            start=(c == 0),
                            stop=(c == 5),
                        )
                    ops.append(op)

                def copy_op(half=half, cell=cell):
                    nc.vector.tensor_scalar_add(
                        qk_sb[:, j, half * QW:(half + 1) * QW],
                        cell["ps"][:],
                        qkb_sb[:, j:j + 1],
                    )
                ops.append(copy_op)
            return ops

        def make_v_fill(t):
            ops = []
            for half in range(2):
                cell = {}
                width = 512 if half == 0 else 256
                for c in range(6):
                    def op(c=c, half=half, width=width, cell=cell):
                        ps = fill_tile(cell)
                        off = 2 * C + half * QW
                        nc.tensor.matmul(
                            ps[:, 0:width],
                            lhsT=xT_sb[:, c, t * P:(t + 1) * P],
                            rhs=w_sb[:, c, off:off + width],
                            start=(c == 0),
                            stop=(c == 5),
                        )
                    ops.append(op)

                def add_op(half=half, width=width, cell=cell):
                    nh0 = half * 8
                    nh = width // HD
                    nc.vector.tensor_add(
                        v_sb[:, t, :].rearrange(
                            "p (h w) -> p h w", w=HD + 1
                        )[:, nh0:nh0 + nh, 0:HD],
                        cell["ps"][:, 0:width].rearrange(
                            "p (h w) -> p h w", w=HD
                        ),
                        vb_sb[:, half * QW:half * QW + width].rearrange(
                            "p (h w) -> p h w", w=HD
                        ),
                    )
                ops.append(add_op)
            return ops

        fill_plan = {
            0: [("v", 6), ("v", 7)],
            1: [("qk", 7), ("qk", 1)],
            2: [("qk", 8)], 3: [("qk", 2)],
            4: [("qk", 9)], 5: [("qk", 3)],
            6: [("qk", 10)], 7: [("qk", 4)],
            8: [("qk", 11)], 9: [("qk", 5)],
            10: [], 11: [],
        }
        fills_by_beat = {}  # global beat -> [callables]
        for u, chunks in fill_plan.items():
            ops = []
            for kind, idx in chunks:
                ops.extend(make_v_fill(idx) if kind == "v" else make_qk_fill(idx))
            if not ops:
                continue
            per_beat = -(-len(ops) // 8)
            for i, op in enumerate(ops):
                g = u * 8 + min(i // per_beat, 7)
                fills_by_beat.setdefault(g, []).append(op)

        def norm_a(u):
            """Evacuate o accumulators + start the reciprocal/broadcast chain."""
            st = unit_state[u]
            ocp = ocpp.tile([P, NTOK], F32, tag="ocp")
            if u == len(units) - 1:
                # the exp chain is over: the idle Scalar engine beats the
                # backed-up DVE queue to free the accumulators that gate the
                # output projection's pair-5 step
                nc.scalar.copy(ocp[0:HD + 1, 0:QW], st["oA"][0:HD + 1, :])
                nc.scalar.copy(ocp[0:HD + 1, QW:NTOK], st["oB"][0:HD + 1, :])
            else:
                nc.vector.tensor_copy(ocp[0:HD + 1, 0:QW], st["oA"][0:HD + 1, :])
                nc.vector.tensor_copy(ocp[0:HD + 1, QW:NTOK], st["oB"][0:HD + 1, :])
            # spread the 1024 sums over 8 partitions so the iterative divide
            # runs 8 lanes wide
            dma = nc.sync if u == len(units) - 1 else nc.gpsimd
            rsp = smallp.tile([32, 32], F32, tag="rsp")
            dma.dma_start(out=rsp[:], in_=ocp[HD:HD + 1, :])
            rinv = smallp.tile([32, 32], F32, tag="rinv")
            nc.vector.reciprocal(rinv[:], rsp[:])
            dscr = dramp.tile([1, NTOK], F32, tag="ds")
            dma.dma_start(out=dscr[:], in_=rinv[:])
            sinv = sinvp.tile([HD, NTOK], F32, tag="sinv")
            dma.dma_start(
                out=sinv[:], in_=dscr[:].broadcast_to((HD, NTOK))
            )
            st["ocp"] = ocp
            st["sinv"] = sinv

        def norm_b(u):
            """Normalize into op_sb (deferred so the DVE never blocks on the
            reciprocal/broadcast latency chain)."""
            p, qt = units[u]
            st = unit_state[u]
            ocp = st["ocp"]
            # the broadcast and the h1 partition-shift ride the idle Sync
            # queue for the last two units so the projection's pair-5 gate
            # isn't stuck behind the congested gpsimd queue
            sinv = st["sinv"]
            dma = nc.sync if u == len(units) - 1 else nc.gpsimd
            nc.vector.tensor_mul(
                op_sb[0:HD, p, qt * QW:(qt + 1) * QW],
                ocp[0:HD, 0:QW],
                sinv[:, 0:QW],
            )
            stage = stgp.tile([HD, QW], BF, tag="stage")
            nc.vector.tensor_mul(stage[:], ocp[0:HD, QW:NTOK], sinv[:, QW:NTOK])
            dma.dma_start(
                out=op_sb[HD:P, p, qt * QW:(qt + 1) * QW], in_=stage[:]
            )

        def av_beat(b):
            u = b // 8
            kcb = b % 8
            if kcb == 0:
                oA = opool.tile([P, QW], F32, tag="o", name="oA")
                oB = opool.tile([P, QW], F32, tag="o", name="oB")
                unit_state[u] = {"oA": oA, "oB": oB}
            st = unit_state[u]
            pt = ptiles.pop(b)
            pu, _ = units[u]
            for h in range(2):
                head = 2 * pu + h
                o_ps = st["oA"] if h == 0 else st["oB"]
                nc.tensor.matmul(
                    o_ps[0:HD + 1, :],
                    lhsT=v_sb[:, kcb, (HD + 1) * head:(HD + 1) * (head + 1)],
                    rhs=pt[:, h * QW:(h + 1) * QW],
                    start=(kcb == 0),
                    stop=(kcb == KC - 1),
                )
            return u if kcb == KC - 1 else None

        for g in range(NBEAT + SKEW):
            for op in deferred.pop(g, ()):
                op()
            if g < NBEAT:
                if g + 2 < NBEAT:
                    issue_e(g + 2)
                score_beat(g)
                for op in fills_by_beat.pop(g, ()):
                    op()
            b = g - SKEW
            if b >= 0:
                udone = av_beat(b)
                if udone is not None:
                    norm_a(udone)
                    deferred.setdefault(g + 5, []).append(
                        lambda u=udone: norm_b(u)
                    )
            if g == 72:
                # proj weights, needed from ~t=150us; issue mid-stream
                nc.sync.dma_start(
                    out=pw_sb[:], in_=pwT_d.rearrange("(c p) m -> p c m", p=P)
                )
                nc.sync.dma_start(out=pb_sb[:], in_=pbT_d[:])
        for ops in deferred.values():
            for op in ops:
                op()

        # ---- output projection ----
        # two groups of 3 psum accumulators; pairs 0-4 accumulate while the
        # last unit's normalization chain drains, pair 5 appended once its
        # op_sb lands, so the PE never idles behind the norm latency
        def proj_alloc_spool():
            ta = spool.tile([P, NTOK], F32, tag="spool", name="projps")
            tb = spool.tile([P, NTOK], F32, tag="spool", name="projps")
            return [
                (ta[:, 0:QW], ta[:, QW:NTOK]),
                (tb[:, 0:QW], tb[:, QW:NTOK]),
            ]

        def proj_partial(ec, halves):
            for p in range(5):
                for nt in range(2):
                    nc.tensor.matmul(
                        halves[nt],
                        lhsT=pw_sb[:, p, ec * P:(ec + 1) * P],
                        rhs=op_sb[:, p, nt * QW:(nt + 1) * QW],
                        start=(p == 0),
                        stop=False,
                    )

        def proj_close(ec, halves, nt):
            nc.tensor.matmul(
                halves[nt],
                lhsT=pw_sb[:, 5, ec * P:(ec + 1) * P],
                rhs=op_sb[:, 5, nt * QW:(nt + 1) * QW],
                start=False,
                stop=True,
            )

        def proj_evac(ec, halves):
            # Scalar and DVE each take one half: both are idle at the tail,
            # halving the bias-add chain; output DMAs alternate queues
            y_sb = youtp.tile([P, NTOK], BF, tag="y", name="y_sb")
            nc.scalar.activation(
                out=y_sb[:, 0:QW], in_=halves[0], func=AF.Identity,
                bias=pb_sb[:, ec:ec + 1],
            )
            nc.vector.tensor_scalar_add(
                y_sb[:, QW:NTOK], halves[1], pb_sb[:, ec:ec + 1]
            )
            dma = nc.sync if ec % 2 == 0 else nc.gpsimd
            dma.dma_start(out=out_d[ec * P:(ec + 1) * P, :], in_=y_sb[:])

        # groups A (spool) and B (the four 1-bank tiles) accumulate pairs 0-4
        # while the last normalization chain drains; every pair-5 closer runs
        # as soon as its op_sb half lands; group C reuses spool afterwards
        ga = proj_alloc_spool()
        for i, ec in enumerate((0, 1)):
            proj_partial(ec, ga[i])
        th0 = qkpool.tile([P, QW], F32, tag="fillps", name="projh0")
        to = [opool.tile([P, QW], F32, tag="o", name="projh") for _ in range(3)]
        gb = [(th0[:], to[0][:]), (to[1][:], to[2][:])]
        for i, ec in enumerate((2, 3)):
            proj_partial(ec, gb[i])
        for nt in range(2):
            for i in range(2):
                proj_close(i, ga[i], nt)
                proj_close(2 + i, gb[i], nt)
        for i in range(2):
            proj_evac(i, ga[i])
            proj_evac(2 + i, gb[i])
        gc = proj_alloc_spool()
        for i, ec in enumerate((4, 5)):
            proj_partial(ec, gc[i])
            for nt in range(2):
                proj_close(ec, gc[i], nt)
            proj_evac(ec, gc[i])


    nc.compile()
    _BUILD_CACHE["nc"] = nc
    return nc


def _prep_inputs(x, qkv_w, q_bias, v_bias, rel_bias_table, proj_w, proj_b,
                 rel_pos_idx):
    x = np.asarray(x, np.float32)
    qkv_w = np.asarray(qkv_w, np.float32)
    q_bias = np.asarray(q_bias, np.float32)
    v_bias = np.asarray(v_bias, np.float32)
    rel_bias_table = np.asarray(rel_bias_table, np.float32)
    proj_w = np.asarray(proj_w, np.float32)
    proj_b = np.asarray(proj_b, np.float32)
    rel_pos_idx = np.asarray(rel_pos_idx, np.int64)

    scale = HD ** -0.5
    wq = qkv_w[:C] * scale
    wqkvT = np.ascontiguousarray(
        np.concatenate([wq, qkv_w[C:]], axis=0).T
    ).astype(BF16NP)

    qk_bias = np.concatenate([q_bias * scale, np.zeros(C, np.float32)])
    qkb = np.ascontiguousarray(qk_bias.reshape(12, P).T)

    vb = v_bias.astype(BF16NP).reshape(1, C)

    # E^T[h, m, n] = exp(bias[h, n, m]); bias[h, n, m] = table[idx[n, m], h]
    A = np.exp(rel_bias_table)[rel_pos_idx]            # (n, m, h)
    ETpre = A.transpose(2, 1, 0)                       # (h, m, n)
    ET = np.ascontiguousarray(
        ETpre.reshape(NPAIR, 2, KC, P, NQT, QW).transpose(0, 4, 2, 3, 1, 5)
    ).astype(BF16NP)

    pwT = np.ascontiguousarray(proj_w.T).astype(BF16NP)
    pbT = np.ascontiguousarray(proj_b.reshape(6, P).T)

    shared = {
        "wqkvT": wqkvT, "qkb": qkb, "vb": vb, "ET": ET,
        "pwT": pwT, "pbT": pbT,
    }
    in_maps = []
    xb16 = x.reshape(NCORES, NTOK, C).astype(BF16NP)
    for b in range(NCORES):
        m = dict(shared)
        m["xT"] = np.ascontiguousarray(xb16[b].T)
        in_maps.append(m)
    return in_maps


def _run(inputs, trace=False):
    import time as _time

    _install_axon_hooks()
    from concourse.bass_utils import run_bass_kernel_spmd

    t0 = _time.time()
    nc = _build()
    print(f"[kernel] build+compile: {_time.time() - t0:.1f}s", flush=True)
    t0 = _time.time()
    in_maps = _prep_inputs(**inputs)
    print(f"[kernel] host prep: {_time.time() - t0:.1f}s", flush=True)
    t0 = _time.time()
    res = run_bass_kernel_spmd(
        nc, in_maps, core_ids=list(range(NCORES)), trace=trace
    )
    print(f"[kernel] hw run: {_time.time() - t0:.1f}s", flush=True)
    outs = [np.asarray(res.results[b]["out"]) for b in range(NCORES)]
    y = np.stack([o.T.reshape(32, 32, C) for o in outs]).astype(np.float32)
    return y, res


def kernel(**inputs) -> np.ndarray:
    y, _ = _run(inputs, trace=False)
    return y


# revision 37
# speedup vs baseline: 1.1440x; 1.0468x over previous
"""BEiT window attention (B=8, N=1024, C=768, 12 heads) on 8 TRN2 NeuronCores.

Sharding: pure data-parallel over batch - one batch element per core, no
collectives.  v2: single software-pipelined emission.

Per-core dataflow (bf16 matmuls, f32 PSUM accumulation): qT/kT computed
feature-major so attention scores are produced directly as S^T (keys on
partitions) and softmax needs no on-device transposes.  P = exp(S^T) * E^T
with E = exp(rel-pos bias) precomputed host-side.  Softmax denominators come
from an all-ones column appended to each V block (row 64 of the attn@v PSUM
accumulators).

Structure (one software-pipelined emission, no serial phases):
 - PE warmup on junk matmuls while input DMAs stream (the PE clock ramps
   0.65->1.2->2.4 GHz with continuous execution).
 - Only qk chunks 0/6 and V chunks 0-5 run up front; V6/V7 and the ten
   remaining qkv chunks are interleaved into the attention beats as PE
   filler (two 1-bank half-accumulations per chunk) so the tensor engine
   never idles while the Scalar engine runs the 96 exp ACTs (~1.11us each).
 - Per beat: scores -> exp (Scalar) -> *E (DVE) -> attn@v, with attn@v
   skewed SKEW beats behind so its semaphores are satisfied on arrival.
 - Softmax reciprocals ride a spread-DMA/32-lane-reciprocal/DRAM-broadcast
   chain; the normalization multiplies are deferred 5 beats so the DVE
   never blocks on that latency; the last unit's chain uses the idle
   Scalar engine + Sync DMA queue since it gates the projection.
 - Output projection in 3 groups of 2 psum accumulators: pairs 0-4
   accumulate while the last normalization drains, pair-5 closers run the
   moment each op_sb half lands.  bf16 DRAM output halves the final drain.
 - PSUM budget exactly 8 banks: scores 2x2, filler 1x1, o-accum 3x1.
"""

import sys
import types

import numpy as np
import ml_dtypes

BF16NP = ml_dtypes.bfloat16

P = 128        # partitions
NTOK = 1024    # tokens per batch element
C = 768        # embed dim
NH = 12        # heads
HD = 64        # head dim
NPAIR = 6      # head pairs
NQT = 2        # query tiles of 512
QW = 512       # query tile width
KC = 8         # key chunks of 128
NCORES = 8
SKEW = 4       # beats between scores and attn@v consumption
NBEAT = NPAIR * NQT * KC  # 96


def _install_axon_hooks():
    """Register the NTFF profile hook module missing from this image's antenv."""
    if "antenv.axon_hooks" in sys.modules:
        return
    try:
        import antenv  # noqa: F401
        from trn_agent_boot.trn_boot import _ntff_profile_via_ctypes

        mod = types.ModuleType("antenv.axon_hooks")
        mod._hook = _ntff_profile_via_ctypes("/opt/axon/libaxon_pjrt.so")
        mod.get_axon_ntff_profile_hook = lambda: mod._hook
        mod.set_axon_ntff_profile_hook = lambda h: setattr(mod, "_hook", h)
        sys.modules["antenv.axon_hooks"] = mod
    except Exception:
        pass


_BUILD_CACHE = {}


def _build():
    if "nc" in _BUILD_CACHE:
        return _BUILD_CACHE["nc"]

    from contextlib import ExitStack

    import concourse.bass as bass
    import concourse.bacc as bacc
    import concourse.mybir as mybir
    import concourse.tile as tile

    BF = mybir.dt.bfloat16
    F32 = mybir.dt.float32
    AF = mybir.ActivationFunctionType

    nc = bacc.Bacc("TRN2", target_bir_lowering=False, debug=False)

    xT_d = nc.dram_tensor("xT", [C, NTOK], BF, kind="ExternalInput").ap()
    wqkvT_d = nc.dram_tensor("wqkvT", [C, 3 * C], BF, kind="ExternalInput").ap()
    qkb_d = nc.dram_tensor("qkb", [P, 12], F32, kind="ExternalInput").ap()
    vb_d = nc.dram_tensor("vb", [1, C], BF, kind="ExternalInput").ap()
    # (pair, qtile, kchunk, key-part, head-in-pair, q)
    ET_d = nc.dram_tensor(
        "ET", [NPAIR, NQT, KC, P, 2, QW], BF, kind="ExternalInput"
    ).ap()
    pwT_d = nc.dram_tensor("pwT", [C, C], BF, kind="ExternalInput").ap()
    pbT_d = nc.dram_tensor("pbT", [P, 6], F32, kind="ExternalInput").ap()
    out_d = nc.dram_tensor("out", [C, NTOK], BF, kind="ExternalOutput").ap()

    units = [(p, qt) for p in range(NPAIR) for qt in range(NQT)]

    with ExitStack() as ctx:
        tc = ctx.enter_context(tile.TileContext(nc))
        const = ctx.enter_context(tc.tile_pool(name="const", bufs=1))
        spool = ctx.enter_context(tc.tile_pool(name="spool", bufs=2, space="PSUM"))
        qkpool = ctx.enter_context(tc.tile_pool(name="qkps", bufs=1, space="PSUM"))
        opool = ctx.enter_context(tc.tile_pool(name="opool", bufs=3, space="PSUM"))
        epool = ctx.enter_context(tc.tile_pool(name="epool", bufs=5))
        prawp = ctx.enter_context(tc.tile_pool(name="praw", bufs=5))
        pfinp = ctx.enter_context(tc.tile_pool(name="pfin", bufs=5))
        ocpp = ctx.enter_context(tc.tile_pool(name="ocp", bufs=2))
        smallp = ctx.enter_context(tc.tile_pool(name="small", bufs=4))
        sinvp = ctx.enter_context(tc.tile_pool(name="sinv", bufs=2))
        stgp = ctx.enter_context(tc.tile_pool(name="stg", bufs=2))
        dramp = ctx.enter_context(tc.tile_pool(name="dram", bufs=4, space="DRAM"))
        youtp = ctx.enter_context(tc.tile_pool(name="yout", bufs=2))

        # ---- persistent SBUF tensors ----
        xT_sb = const.tile([P, 6, NTOK], BF)          # x^T, feature-major
        w_sb = const.tile([P, 6, 3 * C], BF)          # qkv_w^T (q cols pre-scaled)
        qk_sb = const.tile([P, 12, NTOK], BF)         # q^T (chunks 0-5), k^T (6-11)
        v_sb = const.tile([P, KC, NH * (HD + 1)], BF)  # 12x[v_h|1] blocks per kchunk
        op_sb = const.tile([P, NPAIR, NTOK], BF)      # normalized O^T, pair-stacked
        pw_sb = const.tile([P, 6, C], BF)             # proj_w^T
        qkb_sb = const.tile([P, 12], F32)
        pb_sb = const.tile([P, 6], F32)
        vb_sb = const.tile([P, C], BF)

        # ---- input DMAs, priority order (single in-order SP queue) ----
        # half-granularity so the first qk matmuls can start before the full
        # weight set lands
        x_r = xT_d.rearrange("(c p) n -> p c n", p=P)
        w_r = wqkvT_d.rearrange("(c p) m -> p c m", p=P)
        for c in range(6):
            nc.sync.dma_start(out=w_sb[:, c, 0:C], in_=w_r[:, c, 0:C])
            nc.sync.dma_start(out=xT_sb[:, c, :], in_=x_r[:, c, :])
        nc.sync.dma_start(out=qkb_sb[:], in_=qkb_d[:])
        nc.sync.dma_start(out=w_sb[:, 0:3, C:2 * C], in_=w_r[:, 0:3, C:2 * C])
        nc.sync.dma_start(out=w_sb[:, 3:6, C:2 * C], in_=w_r[:, 3:6, C:2 * C])
        nc.sync.dma_start(out=vb_sb[:], in_=vb_d.broadcast_to((P, C)))
        nc.sync.dma_start(out=w_sb[:, 0:3, 2 * C:3 * C], in_=w_r[:, 0:3, 2 * C:3 * C])
        nc.sync.dma_start(out=w_sb[:, 3:6, 2 * C:3 * C], in_=w_r[:, 3:6, 2 * C:3 * C])

        # ones columns of the V blocks (softmax denominator trick)
        nc.gpsimd.memset(
            v_sb[:].rearrange("p k (h w) -> p k h w", w=HD + 1)[:, :, :, HD:HD + 1],
            1.0,
        )

        # PE warmup: the tensor engine clock ramps with continuous execution
        # (0.65 -> 1.2 -> 2.4 GHz after ~3us).  Chew on junk matmuls while the
        # input DMAs stream so the real qkv work starts at full clock.
        junk = const.tile([P, QW], BF)
        nc.gpsimd.memset(junk[:], 0.0)
        for _ in range(20):
            wps = spool.tile([P, NTOK], F32, tag="spool", name="warmps")
            nc.tensor.matmul(
                wps[:, 0:QW], lhsT=junk[:, 0:P], rhs=junk[:],
                start=True, stop=True,
            )

        # ---- building blocks ----
        def qk_matmul_ops(j, pool, cell, tag):
            ops = []

            def get_tile():
                if "ps" not in cell:
                    cell["ps"] = pool.tile(
                        [P, NTOK], F32, tag=tag, name=tag
                    )
                return cell["ps"]

            for c in range(6):
                for half in range(2):
                    def op(c=c, half=half):
                        ps = get_tile()
                        sl = slice(half * QW, (half + 1) * QW)
                        nc.tensor.matmul(
                            ps[:, sl],
                            lhsT=w_sb[:, c, j * P:(j + 1) * P],
                            rhs=xT_sb[:, c, sl],
                            start=(c == 0),
                            stop=(c == 5),
                        )
                    ops.append(op)
            return ops

        def qk_copy_scalar(j, cell):
            # upfront only: Scalar is idle before the exp chain starts
            nc.scalar.activation(
                out=qk_sb[:, j, :], in_=cell["ps"][:], func=AF.Identity,
                bias=qkb_sb[:, j:j + 1],
            )

        def qk_copy_vector(j, cell):
            nc.vector.tensor_scalar_add(
                qk_sb[:, j, :], cell["ps"][:], qkb_sb[:, j:j + 1]
            )

        def v_matmul_ops(t, pool, cell, tag):
            ops = []

            def get_tile():
                if "ps" not in cell:
                    cell["ps"] = pool.tile(
                        [P, NTOK], F32, tag=tag, name=tag
                    )
                return cell["ps"]

            for c in range(6):
                for off, width in ((0, 512), (512, 256)):
                    def op(c=c, off=off, width=width):
                        ps = get_tile()
                        nc.tensor.matmul(
                            ps[:, off:off + width],
                            lhsT=xT_sb[:, c, t * P:(t + 1) * P],
                            rhs=w_sb[:, c, 2 * C + off:2 * C + off + width],
                            start=(c == 0),
                            stop=(c == 5),
                        )
                    ops.append(op)
            return ops

        def v_add(t, cell):
            nc.vector.tensor_add(
                v_sb[:, t, :].rearrange("p (h w) -> p h w", w=HD + 1)[:, :, 0:HD],
                cell["ps"][:, 0:C].rearrange("p (h w) -> p h w", w=HD),
                vb_sb[:].rearrange("p (h w) -> p h w", w=HD),
            )

        # ---- flat attention loop: 96 beats + SKEW trailing ----
        e_tiles = {}
        ptiles = {}
        unit_state = {}
        deferred = {}  # global beat -> [callables]

        def issue_e(g):
            p, qt = units[g // 8]
            kc = g % 8
            e_sb = epool.tile([P, NTOK], BF, tag="e")
            nc.sync.dma_start(
                out=e_sb[:],
                in_=ET_d[p:p + 1, qt:qt + 1, kc:kc + 1, :, :, :].rearrange(
                    "a b c p h q -> (a p) (b c h q)"
                ),
            )
            e_tiles[g] = e_sb

        def score_beat(g):
            p, qt = units[g // 8]
            kc = g % 8
            # scores: S^T tile [128 keys, 2 heads x 512 queries]
            s_ps = spool.tile([P, NTOK], F32, tag="spool", name="s_ps")
            for h in range(2):
                hh = HD * h
                nc.tensor.matmul(
                    s_ps[:, h * QW:(h + 1) * QW],
                    lhsT=qk_sb[hh:hh + HD, 6 + p, kc * P:(kc + 1) * P],
                    rhs=qk_sb[hh:hh + HD, p, qt * QW:(qt + 1) * QW],
                    start=True,
                    stop=True,
                )
            praw = prawp.tile([P, NTOK], BF, tag="praw", name="praw")
            nc.scalar.activation(out=praw[:], in_=s_ps[:], func=AF.Exp)
            ptile = pfinp.tile([P, NTOK], BF, tag="pfin", name="ptile")
            nc.vector.tensor_mul(ptile[:], praw[:], e_tiles.pop(g)[:])
            ptiles[g] = ptile


        # ---- upfront: qk(0), qk(6), V(0..5) ----
        PRE = 0
        for j in (0, 6):
            cell = {}
            for op in qk_matmul_ops(j, spool, cell, "spool"):
                op()
            qk_copy_scalar(j, cell)
        for t in range(6):
            cell = {}
            for op in v_matmul_ops(t, spool, cell, "spool"):
                op()
            v_add(t, cell)
        issue_e(0)
        issue_e(1)

        # ---- fill plan: PE work interleaved into the attention beats ----
        # each chunk is two half-accumulations through a single 1-bank psum
        # tile, so the filler pool costs 1 bank and the freed bank deepens
        # the o-accumulator pool
        def fill_tile(cell):
            if "ps" not in cell:
                cell["ps"] = qkpool.tile(
                    [P, QW], F32, tag="fillps", name="fillps"
                )
            return cell["ps"]

        def make_qk_fill(j):
            ops = []
            for half in range(2):
                cell = {}
                for c in range(6):
                    def op(c=c, half=half, cell=cell):
                        ps = fill_tile(cell)
                        nc.tensor.matmul(
                            ps[:],
                            lhsT=w_sb[:, c, j * P:(j + 1) * P],
                            rhs=xT_sb[:, c, half * QW:(half + 1) * QW],
                            start=(c == 0),
                            stop=(c == 5),
                        )
                    ops.append(op)

                def copy_op(half=half, cell=cell):
                    nc.vector.tensor_scalar_add(
                        qk_sb[:, j, half * QW:(half + 1) * QW],
                        cell["ps"][:],
                        qkb_sb[:, j:j + 1],
                    )
                ops.append(copy_op)
            return ops

        def make_v_fill(t):
            ops = []
            for half in range(2):
                cell = {}
                width = 512 if half == 0 else 256
                for c in range(6):
                    def op(c=c, half=half, width=width, cell=cell):
                        ps = fill_tile(cell)
                        off = 2 * C + half * QW
                        nc.tensor.matmul(
                            ps[:, 0:width],
                            lhsT=xT_sb[:, c, t * P:(t + 1) * P],
                            rhs=w_sb[:, c, off:off + width],
                            start=(c == 0),
                            stop=(c == 5),
                        )
                    ops.append(op)

                def add_op(half=half, width=width, cell=cell):
                    nh0 = half * 8
                    nh = width // HD
                    nc.vector.tensor_add(
                        v_sb[:, t, :].rearrange(
                            "p (h w) -> p h w", w=HD + 1
                        )[:, nh0:nh0 + nh, 0:HD],
                        cell["ps"][:, 0:width].rearrange(
                            "p (h w) -> p h w", w=HD
                        ),
                        vb_sb[:, half * QW:half * QW + width].rearrange(
                            "p (h w) -> p h w", w=HD
                        ),
                    )
                ops.append(add_op)
            return ops

        fill_plan = {
            0: [("v", 6), ("v", 7)],
            1: [("qk", 7), ("qk", 1)],
            2: [("qk", 8)], 3: [("qk", 2)],
            4: [("qk", 9)], 5: [("qk", 3)],
            6: [("qk", 10)], 7: [("qk", 4)],
            8: [("qk", 11)], 9: [("qk", 5)],
            10: [], 11: [],
        }
        fills_by_beat = {}  # global beat -> [callables]
        for u, chunks in fill_plan.items():
            ops = []
            for kind, idx in chunks:
                ops.extend(make_v_fill(idx) if kind == "v" else make_qk_fill(idx))
            if not ops:
                continue
            per_beat = -(-len(ops) // 8)
            for i, op in enumerate(ops):
                g = u * 8 + min(i // per_beat, 7)
                fills_by_beat.setdefault(g, []).append(op)

        def norm_a(u):
            """Evacuate o accumulators + start the reciprocal/broadcast chain."""
            st = unit_state[u]
            ocp = ocpp.tile([P, NTOK], F32, tag="ocp")
            if u == len(units) - 1:
                # the exp chain is over: the idle Scalar engine beats the
                # backed-up DVE queue to free the accumulators that gate the
                # output projection's pair-5 step
                nc.scalar.copy(ocp[0:HD + 1, 0:QW], st["oA"][0:HD + 1, :])
                nc.scalar.copy(ocp[0:HD + 1, QW:NTOK], st["oB"][0:HD + 1, :])
            else:
                nc.vector.tensor_copy(ocp[0:HD + 1, 0:QW], st["oA"][0:HD + 1, :])
                nc.vector.tensor_copy(ocp[0:HD + 1, QW:NTOK], st["oB"][0:HD + 1, :])
            # spread the 1024 sums over 8 partitions so the iterative divide
            # runs 8 lanes wide
            dma = nc.sync if u == len(units) - 1 else nc.gpsimd
            rsp = smallp.tile([32, 32], F32, tag="rsp")
            dma.dma_start(out=rsp[:], in_=ocp[HD:HD + 1, :])
            rinv = smallp.tile([32, 32], F32, tag="rinv")
            nc.vector.reciprocal(rinv[:], rsp[:])
            dscr = dramp.tile([1, NTOK], F32, tag="ds")
            dma.dma_start(out=dscr[:], in_=rinv[:])
            sinv = sinvp.tile([HD, NTOK], F32, tag="sinv")
            dma.dma_start(
                out=sinv[:], in_=dscr[:].broadcast_to((HD, NTOK))
            )
            st["ocp"] = ocp
            st["sinv"] = sinv

        def norm_b(u):
            """Normalize into op_sb (deferred so the DVE never blocks on the
            reciprocal/broadcast latency chain)."""
            p, qt = units[u]
            st = unit_state[u]
            ocp = st["ocp"]
            # the broadcast and the h1 partition-shift ride the idle Sync
            # queue for the last two units so the projection's pair-5 gate
            # isn't stuck behind the congested gpsimd queue
            sinv = st["sinv"]
            dma = nc.sync if u == len(units) - 1 else nc.gpsimd
            nc.vector.tensor_mul(
                op_sb[0:HD, p, qt * QW:(qt + 1) * QW],
                ocp[0:HD, 0:QW],
                sinv[:, 0:QW],
            )
            stage = stgp.tile([HD, QW], BF, tag="stage")
            nc.vector.tensor_mul(stage[:], ocp[0:HD, QW:NTOK], sinv[:, QW:NTOK])
            dma.dma_start(
                out=op_sb[HD:P, p, qt * QW:(qt + 1) * QW], in_=stage[:]
            )

        def av_beat(b):
            u = b // 8
            kcb = b % 8
            if kcb == 0:
                oA = opool.tile([P, QW], F32, tag="o", name="oA")
                oB = opool.tile([P, QW], F32, tag="o", name="oB")
                unit_state[u] = {"oA": oA, "oB": oB}
            st = unit_state[u]
            pt = ptiles.pop(b)
            pu, _ = units[u]
            for h in range(2):
                head = 2 * pu + h
                o_ps = st["oA"] if h == 0 else st["oB"]
                nc.tensor.matmul(
                    o_ps[0:HD + 1, :],
                    lhsT=v_sb[:, kcb, (HD + 1) * head:(HD + 1) * (head + 1)],
                    rhs=pt[:, h * QW:(h + 1) * QW],
                    start=(kcb == 0),
                    stop=(kcb == KC - 1),
                )
            return u if kcb == KC - 1 else None

        for g in range(NBEAT + SKEW):
            for op in deferred.pop(g, ()):
                op()
            if g < NBEAT:
                if g + 2 < NBEAT:
                    issue_e(g + 2)
                score_beat(g)
                for op in fills_by_beat.pop(g, ()):
                    op()
            b = g - SKEW
            if b >= 0:
                udone = av_beat(b)
                if udone is not None:
                    norm_a(udone)
                    deferred.setdefault(g + 5, []).append(
                        lambda u=udone: norm_b(u)
                    )
            if g == 72:
                # proj weights, needed from ~t=150us; issue mid-stream
                nc.sync.dma_start(
                    out=pw_sb[:], in_=pwT_d.rearrange("(c p) m -> p c m", p=P)
                )
                nc.sync.dma_start(out=pb_sb[:], in_=pbT_d[:])
        for ops in deferred.values():
            for op in ops:
                op()

        # ---- output projection ----
        # two groups of 3 psum accumulators; pairs 0-4 accumulate while the
        # last unit's normalization chain drains, pair 5 appended once its
        # op_sb lands, so the PE never idles behind the norm latency
        def proj_alloc_spool():
            ta = spool.tile([P, NTOK], F32, tag="spool", name="projps")
            tb = spool.tile([P, NTOK], F32, tag="spool", name="projps")
            return [
                (ta[:, 0:QW], ta[:, QW:NTOK]),
                (tb[:, 0:QW], tb[:, QW:NTOK]),
            ]

        def proj_partial(ec, halves):
            for p in range(5):
                for nt in range(2):
                    nc.tensor.matmul(
                        halves[nt],
                        lhsT=pw_sb[:, p, ec * P:(ec + 1) * P],
                        rhs=op_sb[:, p, nt * QW:(nt + 1) * QW],
                        start=(p == 0),
                        stop=False,
                    )

        def proj_close(ec, halves, nt):
            nc.tensor.matmul(
                halves[nt],
                lhsT=pw_sb[:, 5, ec * P:(ec + 1) * P],
                rhs=op_sb[:, 5, nt * QW:(nt + 1) * QW],
                start=False,
                stop=True,
            )

        def proj_evac(ec, halves):
            # Scalar and DVE each take one half: both are idle at the tail,
            # halving the bias-add chain; output DMAs alternate queues
            y_sb = youtp.tile([P, NTOK], BF, tag="y", name="y_sb")
            nc.scalar.activation(
                out=y_sb[:, 0:QW], in_=halves[0], func=AF.Identity,
                bias=pb_sb[:, ec:ec + 1],
            )
            nc.vector.tensor_scalar_add(
                y_sb[:, QW:NTOK], halves[1], pb_sb[:, ec:ec + 1]
            )
            dma = nc.sync if ec % 2 == 0 else nc.gpsimd
            dma.dma_start(out=out_d[ec * P:(ec + 1) * P, :], in_=y_sb[:])

        # groups A (spool) and B (the four 1-bank tiles) accumulate pairs 0-4
        # while the last normalization chain drains; every pair-5 closer runs
        # as soon as its op_sb half lands; group C reuses spool afterwards
        ga = proj_alloc_spool()
        for i, ec in enumerate((0, 1)):
            proj_partial(ec, ga[i])
        th0 = qkpool.tile([P, QW], F32, tag="fillps", name="projh0")
        to = [opool.tile([P, QW], F32, tag="o", name="projh") for _ in range(3)]
        gb = [(th0[:], to[0][:]), (to[1][:], to[2][:])]
        for i, ec in enumerate((2, 3)):
            proj_partial(ec, gb[i])
        # keep the PE streaming while the last unit's normalization chain
        # drains (the pair-5 gate): zero-adding junk matmuls into a still-open
        # accumulator hold the clock at full p-state, so the closers and
        # group C run at 2.4GHz instead of re-ramping from 1.2GHz
        def pe_bridge(n):
            for _ in range(n):
                nc.tensor.matmul(
                    ga[0][1], lhsT=junk[:, 0:P], rhs=junk[:],
                    start=False, stop=False,
                )

        pe_bridge(14)
        for nt in range(2):
            for i in range(2):
                proj_close(i, ga[i], nt)
                proj_close(2 + i, gb[i], nt)
            if nt == 0:
                pe_bridge(4)
        for i in range(2):
            proj_evac(i, ga[i])
            proj_evac(2 + i, gb[i])
        gc = proj_alloc_spool()
        for i, ec in enumerate((4, 5)):
            proj_partial(ec, gc[i])
            for nt in range(2):
                proj_close(ec, gc[i], nt)
            proj_evac(ec, gc[i])


    nc.compile()
    _BUILD_CACHE["nc"] = nc
    return nc


def _prep_inputs(x, qkv_w, q_bias, v_bias, rel_bias_table, proj_w, proj_b,
                 rel_pos_idx):
    x = np.asarray(x, np.float32)
    qkv_w = np.asarray(qkv_w, np.float32)
    q_bias = np.asarray(q_bias, np.float32)
    v_bias = np.asarray(v_bias, np.float32)
    rel_bias_table = np.asarray(rel_bias_table, np.float32)
    proj_w = np.asarray(proj_w, np.float32)
    proj_b = np.asarray(proj_b, np.float32)
    rel_pos_idx = np.asarray(rel_pos_idx, np.int64)

    scale = HD ** -0.5
    wq = qkv_w[:C] * scale
    wqkvT = np.ascontiguousarray(
        np.concatenate([wq, qkv_w[C:]], axis=0).T
    ).astype(BF16NP)

    qk_bias = np.concatenate([q_bias * scale, np.zeros(C, np.float32)])
    qkb = np.ascontiguousarray(qk_bias.reshape(12, P).T)

    vb = v_bias.astype(BF16NP).reshape(1, C)

    # E^T[h, m, n] = exp(bias[h, n, m]); bias[h, n, m] = table[idx[n, m], h]
    A = np.exp(rel_bias_table)[rel_pos_idx]            # (n, m, h)
    ETpre = A.transpose(2, 1, 0)                       # (h, m, n)
    ET = np.ascontiguousarray(
        ETpre.reshape(NPAIR, 2, KC, P, NQT, QW).transpose(0, 4, 2, 3, 1, 5)
    ).astype(BF16NP)

    pwT = np.ascontiguousarray(proj_w.T).astype(BF16NP)
    pbT = np.ascontiguousarray(proj_b.reshape(6, P).T)

    shared = {
        "wqkvT": wqkvT, "qkb": qkb, "vb": vb, "ET": ET,
        "pwT": pwT, "pbT": pbT,
    }
    in_maps = []
    xb16 = x.reshape(NCORES, NTOK, C).astype(BF16NP)
    for b in range(NCORES):
        m = dict(shared)
        m["xT"] = np.ascontiguousarray(xb16[b].T)
        in_maps.append(m)
    return in_maps


def _run(inputs, trace=False):
    import time as _time

    _install_axon_hooks()
    from concourse.bass_utils import run_bass_kernel_spmd

    t0 = _time.time()
    nc = _build()
    print(f"[kernel] build+compile: {_time.time() - t0:.1f}s", flush=True)
    t0 = _time.time()
    in_maps = _prep_inputs(**inputs)
    print(f"[kernel] host prep: {_time.time() - t0:.1f}s", flush=True)
    t0 = _time.time()
    res = run_bass_kernel_spmd(
        nc, in_maps, core_ids=list(range(NCORES)), trace=trace
    )
    print(f"[kernel] hw run: {_time.time() - t0:.1f}s", flush=True)
    outs = [np.asarray(res.results[b]["out"]) for b in range(NCORES)]
    y = np.stack([o.T.reshape(32, 32, C) for o in outs]).astype(np.float32)
    return y, res


def kernel(**inputs) -> np.ndarray:
    y, _ = _run(inputs, trace=False)
    return y
